# revision 6
# baseline (speedup 1.0000x reference)
"""Trainium2 Bass kernel for nn_BilinearSentenceEncoder (v3, attention-style).

For sentence [L=128, B=4096, D=300], size [B], W [D, D]:
out[l] = w1*s[l] + w0*s[l-1] + w2*s[l+1] with (w0,w1,w2) a masked 3-way
softmax of bilinear scores s_i^T Wsym s_j / D.

v3 structure (per 16-column chunk, per core; B data-parallel over 8 cores):
  s_t  [128l, 16, 384] bf16   <- SWDGE cast load (HBM f32)
  st   [128d, 16, 3, 128l]    <- XBAR transpose of s_t
  per column j:
    v    = st^T @ (Wsym/D)    3 bf16 matmuls -> PSUM [l, 304]
    v_sb = bf16 copy (Act/DVE alternating)
  vt   [128d, 16, 3, 128l]    <- XBAR transpose of v_sb
  per column j:
    A    = st^T @ vt          3 bf16 matmuls -> PSUM [l', l]  (=scores/D)
    E    = exp(A)             Act -> SBUF bf16 [l', l]
    E'   = E zeroed off-band/oversize (copy_predicated, host u8 mask)
    den  = E'^T @ ones        N=1 matmul -> PSUM [l, 1]; r = 1/den (DVE)
    oraw = E'^T @ s           1 matmul N=300 -> PSUM [l, d]
    o    = oraw * r[l]        Act/DVE alternating -> SBUF f32
  store o chunk (DMA).

Column l of E holds exactly the three channel numerators for output l
(E[l-1,l]=e^{w0 logit}, E[l,l]=e^{w1}, E[l+1,l]=e^{w2}); masking zeroes
invalid channels so the normalized column IS the softmax, and padded
positions degenerate to w1=1 (diag never masked).
"""

import sys

sys.path.insert(0, "/opt/trn_rl_repo")

import numpy as np
import ml_dtypes

import concourse.bacc as bacc
import concourse.mybir as mybir
from concourse import tile
from concourse.bass_utils import run_bass_kernel_spmd

dt = mybir.dt
AF = mybir.ActivationFunctionType
ALU = mybir.AluOpType

L, B, D = 128, 4096, 300
NCORES = 8
BC = B // NCORES          # 512 batch columns per core
CHUNK = 16
NCHUNK = BC // CHUNK      # 32
DP = 384                  # d padded to %128 for the XBAR transpose
DCH = [(0, 128), (128, 128), (256, 44)]


def _build_nc():
    nc = bacc.Bacc()
    f32, bf16, u8 = dt.float32, dt.bfloat16, dt.uint8

    s_in = nc.declare_dram_parameter("s", [L, BC, D], f32, isOutput=False)
    wb_in = nc.declare_dram_parameter("wb", [L, 3, 304], bf16, isOutput=False)
    km_in = nc.declare_dram_parameter("km", [128, BC, 128], u8, isOutput=False)
    o_out = nc.declare_dram_parameter("o", [L, BC, D], f32, isOutput=True)

    with tile.TileContext(nc) as tc:
        with (
            tc.tile_pool(name="const", bufs=1) as cpool,
            tc.tile_pool(name="s", bufs=3) as s_pool,
            tc.tile_pool(name="st", bufs=3) as st_pool,
            tc.tile_pool(name="vsb", bufs=2) as vsb_pool,
            tc.tile_pool(name="vt", bufs=2) as vt_pool,
            tc.tile_pool(name="km", bufs=2) as km_pool,
            tc.tile_pool(name="e", bufs=4) as e_pool,
            tc.tile_pool(name="r", bufs=2) as r_pool,
            tc.tile_pool(name="o", bufs=2) as o_pool,
            tc.tile_pool(name="vp", bufs=2, space="PSUM") as v_pool,
            tc.tile_pool(name="ap", bufs=2, space="PSUM") as a_pool,
            tc.tile_pool(name="dp", bufs=2, space="PSUM") as d_pool,
            tc.tile_pool(name="op", bufs=2, space="PSUM") as ops_pool,
        ):
            wb_t = cpool.tile([L, 3, 304], bf16)
            ones_t = cpool.tile([128, 1], bf16)
            zero_t = cpool.tile([128, 1], bf16)
            nc.sync.dma_start(out=wb_t[:, :, :], in_=wb_in[:, :, :])
            nc.vector.memset(ones_t[:, :], 1.0)
            nc.vector.memset(zero_t[:, :], 0.0)

            for c in range(NCHUNK):
                b0 = c * CHUNK
                s_t = s_pool.tile([L, CHUNK, DP], bf16)
                nc.gpsimd.dma_start(out=s_t[:, :, 0:D], in_=s_in[:, b0 : b0 + CHUNK, :])
                km_t = km_pool.tile([128, CHUNK, 128], u8)
                nc.scalar.dma_start(out=km_t[:, :, :], in_=km_in[:, b0 : b0 + CHUNK, :])

                # XBAR #1: st[p, (j,cc), l] = s_t[l, j, 128*cc + p]
                st = st_pool.tile([128, CHUNK, 3, 128], bf16)
                nc.scalar.dma_start_transpose(st[:, :, :, :], s_t[:, :, :])

                # V: v[l, dout] = sum_d s[l, d] * (Wsym/D)[d, dout]
                v_sb = vsb_pool.tile([L, CHUNK, DP], bf16)
                for j in range(CHUNK):
                    v = v_pool.tile([128, 304], f32)
                    for i, (d0, dn) in enumerate(DCH):
                        nc.tensor.matmul(
                            v[:, :],
                            st[0:dn, j, i, :],
                            wb_t[0:dn, i, :],
                            start=(i == 0),
                            stop=(i == 2),
                        )
                    if j % 2 == 0:
                        nc.scalar.activation(v_sb[:, j, 0:304], v[:, :], AF.Copy)
                    else:
                        nc.vector.tensor_copy(out=v_sb[:, j, 0:304], in_=v[:, :])

                # XBAR #2: vt[p, (j,cc), l] = v_sb[l, j, 128*cc + p]
                vt = vt_pool.tile([128, CHUNK, 3, 128], bf16)
                nc.scalar.dma_start_transpose(vt[:, :, :, :], v_sb[:, :, :])

                o_t = o_pool.tile([L, CHUNK, D], f32)
                r_t = r_pool.tile([128, CHUNK], f32)
                for j in range(CHUNK):
                    # A[l', l] = sum_d s[l', d] v[l, d]  (scores/D; symmetric)
                    a_ps = a_pool.tile([128, 128], f32)
                    for i, (d0, dn) in enumerate(DCH):
                        nc.tensor.matmul(
                            a_ps[:, :],
                            st[0:dn, j, i, :],
                            vt[0:dn, j, i, :],
                            start=(i == 0),
                            stop=(i == 2),
                        )
                    e_t = e_pool.tile([128, 128], bf16)
                    nc.scalar.activation(e_t[:, :], a_ps[:, :], AF.Exp)
                    # zero everything except allowed band cells (kill-mask=1)
                    zb = zero_t[:, :].broadcast_to([128, 128])
                    nc.vector.copy_predicated(
                        out=e_t[:, :], mask=km_t[:, j, :], data=zb
                    )
                    # den[l] = sum_l' E'[l', l];  r = 1/den
                    den = d_pool.tile([128, 1], f32)
                    nc.tensor.matmul(
                        den[:, :], e_t[:, :], ones_t[:, :], start=True, stop=True
                    )
                    nc.vector.reciprocal(r_t[:, j : j + 1], den[:, :])
                    # oraw[l, d] = sum_l' E'[l', l] s[l', d]
                    ops = ops_pool.tile([128, D], f32)
                    nc.tensor.matmul(
                        ops[:, :], e_t[:, :], s_t[:, j, 0:D], start=True, stop=True
                    )
                    if j % 2 == 0:
                        nc.scalar.activation(
                            o_t[:, j, :], ops[:, :], AF.Copy,
                            scale=r_t[:, j : j + 1],
                        )
                    else:
                        nc.vector.tensor_scalar(
                            out=o_t[:, j, :], in0=ops[:, :],
                            scalar1=r_t[:, j : j + 1], scalar2=None,
                            op0=ALU.mult,
                        )
                nc.sync.dma_start(out=o_out[:, b0 : b0 + CHUNK, :], in_=o_t[:, :, :])

    nc.compile()
    return nc


_NC_CACHE = {}


def _get_nc():
    if "nc" not in _NC_CACHE:
        _NC_CACHE["nc"] = _build_nc()
    return _NC_CACHE["nc"]


def _host_inputs(sentence, size, W):
    sentence = np.ascontiguousarray(np.asarray(sentence, dtype=np.float32))
    size = np.asarray(size).astype(np.int64)
    W = np.asarray(W, dtype=np.float32)

    wsym = 0.5 * (W + W.T) / np.float32(D)
    wb = np.zeros((128, 3, 304), dtype=ml_dtypes.bfloat16)
    for i, (d0, dn) in enumerate(DCH):
        wb[0:dn, i, 0:D] = wsym[d0 : d0 + dn, :].astype(ml_dtypes.bfloat16)

    # kill-mask km[l', b, l] = 1 where E must be zeroed.
    # allowed cells per output column l: (l, l) always;
    # (l-1, l) iff l >= 1 and l < size_b; (l+1, l) iff l <= 126 and l < size_b - 1.
    lp = np.arange(128)[:, None, None]          # l'
    lc = np.arange(128)[None, None, :]          # l
    sz = size[None, :, None].astype(np.int64)   # b
    allow = (lp == lc)
    allow = allow | ((lp == lc - 1) & (lc < sz))
    allow = allow | ((lp == lc + 1) & (lc < sz - 1))
    km_full = (~allow).astype(np.uint8)         # [128, B, 128]

    in_maps = []
    for c in range(NCORES):
        bsl = slice(c * BC, (c + 1) * BC)
        in_maps.append(
            {
                "s": np.ascontiguousarray(sentence[:, bsl, :]),
                "wb": wb,
                "km": np.ascontiguousarray(km_full[:, bsl, :]),
            }
        )
    return in_maps


def kernel(sentence, size, W):
    nc = _get_nc()
    in_maps = _host_inputs(sentence, size, W)
    res = run_bass_kernel_spmd(nc, in_maps, core_ids=list(range(NCORES)))
    out = np.concatenate([res.results[c]["o"] for c in range(NCORES)], axis=1)
    return out.astype(np.float32)


def _install_ntff_hook():
    """Register the axon NTFF profiling hook that this container's boot
    skipped (antenv.axon_hooks module absent)."""
    try:
        from antenv.axon_hooks import get_axon_ntff_profile_hook  # noqa: F401

        return
    except ImportError:
        pass
    import contextlib
    import ctypes
    import types

    so_path = "/opt/axon/libaxon_pjrt.so"
    lib = ctypes.CDLL(so_path)
    if not hasattr(lib, "axon_start_nrt_profile"):
        return
    lib.axon_start_nrt_profile.argtypes = [
        ctypes.POINTER(ctypes.c_int64),
        ctypes.c_size_t,
    ]
    lib.axon_start_nrt_profile.restype = ctypes.c_int64
    lib.axon_stop_nrt_profile.argtypes = [ctypes.c_char_p]
    lib.axon_stop_nrt_profile.restype = ctypes.c_int64

    @contextlib.contextmanager
    def _hook(output_dir, device_ids):
        import jax

        jax.devices()
        if device_ids:
            ids = (ctypes.c_int64 * len(device_ids))(*device_ids)
            rc = lib.axon_start_nrt_profile(ids, len(device_ids))
        else:
            rc = lib.axon_start_nrt_profile(None, 0)
        if rc != 0:
            raise RuntimeError(f"axon_start_nrt_profile rc={rc}")
        try:
            yield
        finally:
            n = lib.axon_stop_nrt_profile(str(output_dir).encode())
            print(f"ntff capture: {n} file(s) -> {output_dir}")

    mod = types.ModuleType("antenv.axon_hooks")
    mod.get_axon_ntff_profile_hook = lambda: _hook
    mod.set_axon_ntff_profile_hook = lambda h: None
    import antenv

    sys.modules["antenv.axon_hooks"] = mod
    antenv.axon_hooks = mod


def run_traced(sentence, size, W):
    """Like kernel(), but also returns (exec_time_ns, profile_json path)."""
    _install_ntff_hook()
    nc = _get_nc()
    in_maps = _host_inputs(sentence, size, W)
    res = run_bass_kernel_spmd(
        nc, in_maps, core_ids=list(range(NCORES)), trace=True, trace_cores=[0]
    )
    out = np.concatenate([res.results[c]["o"] for c in range(NCORES)], axis=1)
    return out.astype(np.float32), res.exec_time_ns, res.profile_json


if __name__ == "__main__":
    rng = np.random.default_rng(0)
    s = rng.standard_normal((L, B, D)).astype(np.float32)
    sz = rng.integers(0, L, size=(B,)).astype(np.int32)
    W = (rng.standard_normal((D, D)) / np.sqrt(D)).astype(np.float32)
    out = kernel(s, sz, W)
    print("out", out.shape, out.dtype, np.abs(out).max())


# revision 8
# speedup vs baseline: 1.2133x; 1.2133x over previous
"""Trainium2 Bass kernel for nn_BilinearSentenceEncoder (v3, attention-style).

For sentence [L=128, B=4096, D=300], size [B], W [D, D]:
out[l] = w1*s[l] + w0*s[l-1] + w2*s[l+1] with (w0,w1,w2) a masked 3-way
softmax of bilinear scores s_i^T Wsym s_j / D.

v3 structure (per 16-column chunk, per core; B data-parallel over 8 cores):
  s_t  [128l, 16, 384] bf16   <- SWDGE cast load (HBM f32)
  st   [128d, 16, 3, 128l]    <- XBAR transpose of s_t
  per column j:
    v    = st^T @ (Wsym/D)    3 bf16 matmuls -> PSUM [l, 304]
    v_sb = bf16 copy (Act/DVE alternating)
  vt   [128d, 16, 3, 128l]    <- XBAR transpose of v_sb
  per column j:
    A    = st^T @ vt          3 bf16 matmuls -> PSUM [l', l]  (=scores/D)
    E    = exp(A)             Act -> SBUF bf16 [l', l]
    E'   = E zeroed off-band/oversize (copy_predicated, host u8 mask)
    den  = E'^T @ ones        N=1 matmul -> PSUM [l, 1]; r = 1/den (DVE)
    oraw = E'^T @ s           1 matmul N=300 -> PSUM [l, d]
    o    = oraw * r[l]        Act/DVE alternating -> SBUF f32
  store o chunk (DMA).

Column l of E holds exactly the three channel numerators for output l
(E[l-1,l]=e^{w0 logit}, E[l,l]=e^{w1}, E[l+1,l]=e^{w2}); masking zeroes
invalid channels so the normalized column IS the softmax, and padded
positions degenerate to w1=1 (diag never masked).
"""

import sys

sys.path.insert(0, "/opt/trn_rl_repo")

import numpy as np
import ml_dtypes

import concourse.bacc as bacc
import concourse.mybir as mybir
from concourse import tile
from concourse.bass_utils import run_bass_kernel_spmd

dt = mybir.dt
AF = mybir.ActivationFunctionType
ALU = mybir.AluOpType

L, B, D = 128, 4096, 300
NCORES = 8
BC = B // NCORES          # 512 batch columns per core
CHUNK = 16
NCHUNK = BC // CHUNK      # 32
DP = 384                  # d padded to %128 for the XBAR transpose
DCH = [(0, 128), (128, 128), (256, 44)]


def _build_nc():
    nc = bacc.Bacc()
    f32, bf16, u8 = dt.float32, dt.bfloat16, dt.uint8

    s_in = nc.declare_dram_parameter("s", [L, BC, D], f32, isOutput=False)
    wb_in = nc.declare_dram_parameter("wb", [L, 3, 304], bf16, isOutput=False)
    km_in = nc.declare_dram_parameter("km", [128, BC, 128], u8, isOutput=False)
    o_out = nc.declare_dram_parameter("o", [L, BC, D], f32, isOutput=True)

    with tile.TileContext(nc) as tc:
        with (
            tc.tile_pool(name="const", bufs=1) as cpool,
            tc.tile_pool(name="s", bufs=3) as s_pool,
            tc.tile_pool(name="st", bufs=3) as st_pool,
            tc.tile_pool(name="vt", bufs=4) as vt_pool,
            tc.tile_pool(name="km", bufs=2) as km_pool,
            tc.tile_pool(name="e", bufs=2) as e_pool,
            tc.tile_pool(name="r", bufs=2) as r_pool,
            tc.tile_pool(name="o", bufs=2) as o_pool,
            tc.tile_pool(name="vp", bufs=2, space="PSUM") as v_pool,
            tc.tile_pool(name="ap", bufs=2, space="PSUM") as a_pool,
            tc.tile_pool(name="dp", bufs=2, space="PSUM") as d_pool,
            tc.tile_pool(name="op", bufs=2, space="PSUM") as ops_pool,
        ):
            wb_t = cpool.tile([L, 3, 304], bf16)
            ones_t = cpool.tile([128, 1], bf16)
            zero_t = cpool.tile([128, 1], bf16)
            nc.sync.dma_start(out=wb_t[:, :, :], in_=wb_in[:, :, :])
            nc.vector.memset(ones_t[:, :], 1.0)
            nc.vector.memset(zero_t[:, :], 0.0)

            MCH = [(0, 128), (128, 128), (256, 44)]   # dout chunks
            for c in range(NCHUNK):
                b0 = c * CHUNK
                s_t = s_pool.tile([L, CHUNK, DP], bf16)
                nc.gpsimd.dma_start(out=s_t[:, :, 0:D], in_=s_in[:, b0 : b0 + CHUNK, :])
                km_t = km_pool.tile([128, CHUNK, 128], u8)
                nc.gpsimd.dma_start(out=km_t[:, :, :], in_=km_in[:, b0 : b0 + CHUNK, :])

                # XBAR: st[p, (j,cc), l] = s_t[l, j, 128*cc + p]
                st = st_pool.tile([128, CHUNK, 3, 128], bf16)
                nc.sync.dma_start_transpose(st[:, :, :, :], s_t[:, :, :])

                e_ch = e_pool.tile([128, CHUNK, 128], bf16)
                for j in range(CHUNK):
                    # vt[dout, l] = sum_din (Wsym/D)[din, dout] s[l, din]
                    vt_ps = v_pool.tile([128, 3, 128], f32)
                    for mi, (m0, mn) in enumerate(MCH):
                        for k, (d0, dn) in enumerate(DCH):
                            nc.tensor.matmul(
                                vt_ps[0:mn, mi, :],
                                wb_t[0:dn, k, m0 : m0 + mn],
                                st[0:dn, j, k, :],
                                start=(k == 0),
                                stop=(k == 2),
                            )
                    vt_sb = vt_pool.tile([128, 3, 128], bf16)
                    if j % 2 == 0:
                        nc.scalar.activation(vt_sb[:, :, :], vt_ps[:, :, :], AF.Copy)
                    else:
                        nc.vector.tensor_copy(out=vt_sb[:, :, :], in_=vt_ps[:, :, :])
                    # A[l', l] = sum_dout s[l', dout] vt[dout, l]  (scores/D)
                    a_ps = a_pool.tile([128, 128], f32)
                    for k2, (d0, dn) in enumerate(MCH):
                        nc.tensor.matmul(
                            a_ps[:, :],
                            st[0:dn, j, k2, :],
                            vt_sb[0:dn, k2, :],
                            start=(k2 == 0),
                            stop=(k2 == 2),
                        )
                    nc.scalar.activation(e_ch[:, j, :], a_ps[:, :], AF.Exp)

                # zero non-band / oversize cells (kill-mask=1), half-chunks
                zb = zero_t[:, :].broadcast_to([128, 8, 128])
                nc.vector.copy_predicated(
                    out=e_ch[:, 0:8, :], mask=km_t[:, 0:8, :], data=zb
                )
                nc.vector.copy_predicated(
                    out=e_ch[:, 8:16, :], mask=km_t[:, 8:16, :], data=zb
                )

                # den[l] = sum_l' E'[l', l] per column, batched recip per chunk
                den_ch = d_pool.tile([128, CHUNK], f32)
                for j in range(CHUNK):
                    nc.tensor.matmul(
                        den_ch[:, j : j + 1], e_ch[:, j, :], ones_t[:, :],
                        start=True, stop=True,
                    )
                r_t = r_pool.tile([128, CHUNK], f32)
                nc.vector.reciprocal(r_t[:, :], den_ch[:, :])

                o_t = o_pool.tile([L, CHUNK, D], f32)
                for j in range(CHUNK):
                    # oraw[l, d] = sum_l' E'[l', l] s[l', d]
                    ops = ops_pool.tile([128, D], f32)
                    nc.tensor.matmul(
                        ops[:, :], e_ch[:, j, :], s_t[:, j, 0:D], start=True, stop=True
                    )
                    if j % 2 == 0:
                        nc.scalar.activation(
                            o_t[:, j, :], ops[:, :], AF.Copy,
                            scale=r_t[:, j : j + 1],
                        )
                    else:
                        nc.vector.tensor_scalar(
                            out=o_t[:, j, :], in0=ops[:, :],
                            scalar1=r_t[:, j : j + 1], scalar2=None,
                            op0=ALU.mult,
                        )
                nc.sync.dma_start(out=o_out[:, b0 : b0 + CHUNK, :], in_=o_t[:, :, :])

    nc.compile()
    return nc


_NC_CACHE = {}


def _get_nc():
    if "nc" not in _NC_CACHE:
        _NC_CACHE["nc"] = _build_nc()
    return _NC_CACHE["nc"]


def _host_inputs(sentence, size, W):
    sentence = np.ascontiguousarray(np.asarray(sentence, dtype=np.float32))
    size = np.asarray(size).astype(np.int64)
    W = np.asarray(W, dtype=np.float32)

    wsym = 0.5 * (W + W.T) / np.float32(D)
    wb = np.zeros((128, 3, 304), dtype=ml_dtypes.bfloat16)
    for i, (d0, dn) in enumerate(DCH):
        wb[0:dn, i, 0:D] = wsym[d0 : d0 + dn, :].astype(ml_dtypes.bfloat16)

    # kill-mask km[l', b, l] = 1 where E must be zeroed.
    # allowed cells per output column l: (l, l) always;
    # (l-1, l) iff l >= 1 and l < size_b; (l+1, l) iff l <= 126 and l < size_b - 1.
    lp = np.arange(128)[:, None, None]          # l'
    lc = np.arange(128)[None, None, :]          # l
    sz = size[None, :, None].astype(np.int64)   # b
    allow = (lp == lc)
    allow = allow | ((lp == lc - 1) & (lc < sz))
    allow = allow | ((lp == lc + 1) & (lc < sz - 1))
    km_full = (~allow).astype(np.uint8)         # [128, B, 128]

    in_maps = []
    for c in range(NCORES):
        bsl = slice(c * BC, (c + 1) * BC)
        in_maps.append(
            {
                "s": np.ascontiguousarray(sentence[:, bsl, :]),
                "wb": wb,
                "km": np.ascontiguousarray(km_full[:, bsl, :]),
            }
        )
    return in_maps


def kernel(sentence, size, W):
    nc = _get_nc()
    in_maps = _host_inputs(sentence, size, W)
    res = run_bass_kernel_spmd(nc, in_maps, core_ids=list(range(NCORES)))
    out = np.concatenate([res.results[c]["o"] for c in range(NCORES)], axis=1)
    return out.astype(np.float32)


def _install_ntff_hook():
    """Register the axon NTFF profiling hook that this container's boot
    skipped (antenv.axon_hooks module absent)."""
    try:
        from antenv.axon_hooks import get_axon_ntff_profile_hook  # noqa: F401

        return
    except ImportError:
        pass
    import contextlib
    import ctypes
    import types

    so_path = "/opt/axon/libaxon_pjrt.so"
    lib = ctypes.CDLL(so_path)
    if not hasattr(lib, "axon_start_nrt_profile"):
        return
    lib.axon_start_nrt_profile.argtypes = [
        ctypes.POINTER(ctypes.c_int64),
        ctypes.c_size_t,
    ]
    lib.axon_start_nrt_profile.restype = ctypes.c_int64
    lib.axon_stop_nrt_profile.argtypes = [ctypes.c_char_p]
    lib.axon_stop_nrt_profile.restype = ctypes.c_int64

    @contextlib.contextmanager
    def _hook(output_dir, device_ids):
        import jax

        jax.devices()
        if device_ids:
            ids = (ctypes.c_int64 * len(device_ids))(*device_ids)
            rc = lib.axon_start_nrt_profile(ids, len(device_ids))
        else:
            rc = lib.axon_start_nrt_profile(None, 0)
        if rc != 0:
            raise RuntimeError(f"axon_start_nrt_profile rc={rc}")
        try:
            yield
        finally:
            n = lib.axon_stop_nrt_profile(str(output_dir).encode())
            print(f"ntff capture: {n} file(s) -> {output_dir}")

    mod = types.ModuleType("antenv.axon_hooks")
    mod.get_axon_ntff_profile_hook = lambda: _hook
    mod.set_axon_ntff_profile_hook = lambda h: None
    import antenv

    sys.modules["antenv.axon_hooks"] = mod
    antenv.axon_hooks = mod


def run_traced(sentence, size, W):
    """Like kernel(), but also returns (exec_time_ns, profile_json path)."""
    _install_ntff_hook()
    nc = _get_nc()
    in_maps = _host_inputs(sentence, size, W)
    res = run_bass_kernel_spmd(
        nc, in_maps, core_ids=list(range(NCORES)), trace=True, trace_cores=[0]
    )
    out = np.concatenate([res.results[c]["o"] for c in range(NCORES)], axis=1)
    return out.astype(np.float32), res.exec_time_ns, res.profile_json


if __name__ == "__main__":
    rng = np.random.default_rng(0)
    s = rng.standard_normal((L, B, D)).astype(np.float32)
    sz = rng.integers(0, L, size=(B,)).astype(np.int32)
    W = (rng.standard_normal((D, D)) / np.sqrt(D)).astype(np.float32)
    out = kernel(s, sz, W)
    print("out", out.shape, out.dtype, np.abs(out).max())


# revision 11
# speedup vs baseline: 1.3537x; 1.1157x over previous
"""Trainium2 Bass kernel for nn_BilinearSentenceEncoder (v3, attention-style).

For sentence [L=128, B=4096, D=300], size [B], W [D, D]:
out[l] = w1*s[l] + w0*s[l-1] + w2*s[l+1] with (w0,w1,w2) a masked 3-way
softmax of bilinear scores s_i^T Wsym s_j / D.

v3 structure (per 16-column chunk, per core; B data-parallel over 8 cores):
  s_t  [128l, 16, 384] bf16   <- SWDGE cast load (HBM f32)
  st   [128d, 16, 3, 128l]    <- XBAR transpose of s_t
  per column j:
    v    = st^T @ (Wsym/D)    3 bf16 matmuls -> PSUM [l, 304]
    v_sb = bf16 copy (Act/DVE alternating)
  vt   [128d, 16, 3, 128l]    <- XBAR transpose of v_sb
  per column j:
    A    = st^T @ vt          3 bf16 matmuls -> PSUM [l', l]  (=scores/D)
    E    = exp(A)             Act -> SBUF bf16 [l', l]
    E'   = E zeroed off-band/oversize (copy_predicated, host u8 mask)
    den  = E'^T @ ones        N=1 matmul -> PSUM [l, 1]; r = 1/den (DVE)
    oraw = E'^T @ s           1 matmul N=300 -> PSUM [l, d]
    o    = oraw * r[l]        Act/DVE alternating -> SBUF f32
  store o chunk (DMA).

Column l of E holds exactly the three channel numerators for output l
(E[l-1,l]=e^{w0 logit}, E[l,l]=e^{w1}, E[l+1,l]=e^{w2}); masking zeroes
invalid channels so the normalized column IS the softmax, and padded
positions degenerate to w1=1 (diag never masked).
"""

import sys

sys.path.insert(0, "/opt/trn_rl_repo")

import numpy as np
import ml_dtypes

import concourse.bacc as bacc
import concourse.mybir as mybir
from concourse import tile
from concourse.bass_utils import run_bass_kernel_spmd

dt = mybir.dt
AF = mybir.ActivationFunctionType
ALU = mybir.AluOpType

L, B, D = 128, 4096, 300
NCORES = 8
BC = B // NCORES          # 512 batch columns per core
CHUNK = 16
NCHUNK = BC // CHUNK      # 32
DP = 384                  # d padded to %128 for the XBAR transpose
DCH = [(0, 128), (128, 128), (256, 44)]


def _build_nc():
    nc = bacc.Bacc()
    f32, bf16, u8 = dt.float32, dt.bfloat16, dt.uint8

    s_in = nc.declare_dram_parameter("s", [L, BC, D], f32, isOutput=False)
    wb_in = nc.declare_dram_parameter("wb", [L, 3, 304], bf16, isOutput=False)
    km_in = nc.declare_dram_parameter("km", [128, BC, 128], u8, isOutput=False)
    o_out = nc.declare_dram_parameter("o", [L, BC, D], f32, isOutput=True)

    with tile.TileContext(nc) as tc:
        with (
            tc.tile_pool(name="const", bufs=1) as cpool,
            tc.tile_pool(name="s", bufs=3) as s_pool,
            tc.tile_pool(name="st", bufs=3) as st_pool,
            tc.tile_pool(name="vt", bufs=4) as vt_pool,
            tc.tile_pool(name="km", bufs=2) as km_pool,
            tc.tile_pool(name="e", bufs=2) as e_pool,
            tc.tile_pool(name="r", bufs=2) as r_pool,
            tc.tile_pool(name="o", bufs=2) as o_pool,
            tc.tile_pool(name="vp", bufs=1, space="PSUM") as v_pool,
            tc.tile_pool(name="ap", bufs=2, space="PSUM") as a_pool,
            tc.tile_pool(name="dp", bufs=2, space="PSUM") as d_pool,
            tc.tile_pool(name="op", bufs=2, space="PSUM") as ops_pool,
        ):
            wb_t = cpool.tile([L, 3, 304], bf16)
            ones_t = cpool.tile([128, 1], bf16)
            zero_t = cpool.tile([128, 1], bf16)
            nc.sync.dma_start(out=wb_t[:, :, :], in_=wb_in[:, :, :])
            nc.vector.memset(ones_t[:, :], 1.0)
            nc.vector.memset(zero_t[:, :], 0.0)

            MCH = [(0, 128), (128, 128), (256, 44)]   # dout chunks
            for c in range(NCHUNK):
                b0 = c * CHUNK
                s_t = s_pool.tile([L, CHUNK, DP], bf16)
                nc.gpsimd.dma_start(out=s_t[:, :, 0:D], in_=s_in[:, b0 : b0 + CHUNK, :])
                # ones column at d=300 feeds the fused den (col 300 of oraw)
                nc.vector.memset(s_t[:, :, 300:301], 1.0)
                km_t = km_pool.tile([128, CHUNK, 128], u8)
                nc.gpsimd.dma_start(out=km_t[:, :, :], in_=km_in[:, b0 : b0 + CHUNK, :])

                # XBAR: st[p, (j,cc), l] = s_t[l, j, 128*cc + p]
                st = st_pool.tile([128, CHUNK, 3, 128], bf16)
                nc.sync.dma_start_transpose(st[:, :, :, :], s_t[:, :, :])

                e_ch = e_pool.tile([128, CHUNK, 128], bf16)
                o_t = o_pool.tile([L, CHUNK, D], f32)
                r_t = r_pool.tile([128, CHUNK], f32)
                zb = zero_t[:, :].broadcast_to([128, 8, 128])

                def emit_quarter(q):
                    j0 = 4 * q
                    # vt[dout, j, l] = sum_din (Wsym/D)[din, dout] s[l, j, din]
                    # moving batched over 4 columns (N=512, PSUM-bank limit)
                    vt_sb = vt_pool.tile([128, 4, 3, 128], bf16)
                    vt_ps = v_pool.tile([128, 3, 4, 128], f32)
                    for mi, (m0, mn) in enumerate(MCH):
                        for k, (d0, dn) in enumerate(DCH):
                            nc.tensor.matmul(
                                vt_ps[0:mn, mi, :, :],
                                wb_t[0:dn, k, m0 : m0 + mn],
                                st[0:dn, j0 : j0 + 4, k, :],
                                start=(k == 0),
                                stop=(k == 2),
                            )
                    if q % 2 == 0:
                        nc.scalar.activation(
                            vt_sb[:, :, :, :],
                            vt_ps[:, :, :, :].rearrange("p m j l -> p j m l"),
                            AF.Copy,
                        )
                    else:
                        nc.vector.tensor_copy(
                            out=vt_sb[:, :, :, :],
                            in_=vt_ps[:, :, :, :].rearrange("p m j l -> p j m l"),
                        )
                    for jh in range(4):
                        j = j0 + jh
                        # A[l', l] = sum_dout s[l', dout] vt[dout, l] (scores/D)
                        a_ps = a_pool.tile([128, 128], f32)
                        for k2, (d0, dn) in enumerate(MCH):
                            nc.tensor.matmul(
                                a_ps[:, :],
                                st[0:dn, j, k2, :],
                                vt_sb[0:dn, jh, k2, :],
                                start=(k2 == 0),
                                stop=(k2 == 2),
                            )
                        nc.scalar.activation(e_ch[:, j, :], a_ps[:, :], AF.Exp)

                def emit_mask(h):
                    j0 = 8 * h
                    nc.vector.copy_predicated(
                        out=e_ch[:, j0 : j0 + 8, :], mask=km_t[:, j0 : j0 + 8, :],
                        data=zb,
                    )

                def emit_out_half(h):
                    j0 = 8 * h
                    for jh in range(8):
                        j = j0 + jh
                        # oraw[l, 0:300] = sum_l' E'[l', l] s[l', d];  col 300 = den
                        ops = ops_pool.tile([128, 301], f32)
                        nc.tensor.matmul(
                            ops[:, :], e_ch[:, j, :], s_t[:, j, 0:301],
                            start=True, stop=True,
                        )
                        nc.vector.reciprocal(r_t[:, j : j + 1], ops[:, 300:301])
                        if j % 2 == 0:
                            nc.scalar.activation(
                                o_t[:, j, :], ops[:, 0:300], AF.Copy,
                                scale=r_t[:, j : j + 1],
                            )
                        else:
                            nc.vector.tensor_scalar(
                                out=o_t[:, j, :], in0=ops[:, 0:300],
                                scalar1=r_t[:, j : j + 1], scalar2=None,
                                op0=ALU.mult,
                            )

                emit_quarter(0)
                emit_quarter(1)
                emit_mask(0)
                emit_quarter(2)
                emit_out_half(0)
                emit_quarter(3)
                emit_mask(1)
                emit_out_half(1)
                nc.sync.dma_start(out=o_out[:, b0 : b0 + CHUNK, :], in_=o_t[:, :, :])

    nc.compile()
    return nc


_NC_CACHE = {}


def _get_nc():
    if "nc" not in _NC_CACHE:
        _NC_CACHE["nc"] = _build_nc()
    return _NC_CACHE["nc"]


def _host_inputs(sentence, size, W):
    sentence = np.ascontiguousarray(np.asarray(sentence, dtype=np.float32))
    size = np.asarray(size).astype(np.int64)
    W = np.asarray(W, dtype=np.float32)

    wsym = 0.5 * (W + W.T) / np.float32(D)
    wb = np.zeros((128, 3, 304), dtype=ml_dtypes.bfloat16)
    for i, (d0, dn) in enumerate(DCH):
        wb[0:dn, i, 0:D] = wsym[d0 : d0 + dn, :].astype(ml_dtypes.bfloat16)

    # kill-mask km[l', b, l] = 1 where E must be zeroed.
    # allowed cells per output column l: (l, l) always;
    # (l-1, l) iff l >= 1 and l < size_b; (l+1, l) iff l <= 126 and l < size_b - 1.
    lp = np.arange(128)[:, None, None]          # l'
    lc = np.arange(128)[None, None, :]          # l
    sz = size[None, :, None].astype(np.int64)   # b
    allow = (lp == lc)
    allow = allow | ((lp == lc - 1) & (lc < sz))
    allow = allow | ((lp == lc + 1) & (lc < sz - 1))
    km_full = (~allow).astype(np.uint8)         # [128, B, 128]

    in_maps = []
    for c in range(NCORES):
        bsl = slice(c * BC, (c + 1) * BC)
        in_maps.append(
            {
                "s": np.ascontiguousarray(sentence[:, bsl, :]),
                "wb": wb,
                "km": np.ascontiguousarray(km_full[:, bsl, :]),
            }
        )
    return in_maps


def kernel(sentence, size, W):
    nc = _get_nc()
    in_maps = _host_inputs(sentence, size, W)
    res = run_bass_kernel_spmd(nc, in_maps, core_ids=list(range(NCORES)))
    out = np.concatenate([res.results[c]["o"] for c in range(NCORES)], axis=1)
    return out.astype(np.float32)


def _install_ntff_hook():
    """Register the axon NTFF profiling hook that this container's boot
    skipped (antenv.axon_hooks module absent)."""
    try:
        from antenv.axon_hooks import get_axon_ntff_profile_hook  # noqa: F401

        return
    except ImportError:
        pass
    import contextlib
    import ctypes
    import types

    so_path = "/opt/axon/libaxon_pjrt.so"
    lib = ctypes.CDLL(so_path)
    if not hasattr(lib, "axon_start_nrt_profile"):
        return
    lib.axon_start_nrt_profile.argtypes = [
        ctypes.POINTER(ctypes.c_int64),
        ctypes.c_size_t,
    ]
    lib.axon_start_nrt_profile.restype = ctypes.c_int64
    lib.axon_stop_nrt_profile.argtypes = [ctypes.c_char_p]
    lib.axon_stop_nrt_profile.restype = ctypes.c_int64

    @contextlib.contextmanager
    def _hook(output_dir, device_ids):
        import jax

        jax.devices()
        if device_ids:
            ids = (ctypes.c_int64 * len(device_ids))(*device_ids)
            rc = lib.axon_start_nrt_profile(ids, len(device_ids))
        else:
            rc = lib.axon_start_nrt_profile(None, 0)
        if rc != 0:
            raise RuntimeError(f"axon_start_nrt_profile rc={rc}")
        try:
            yield
        finally:
            n = lib.axon_stop_nrt_profile(str(output_dir).encode())
            print(f"ntff capture: {n} file(s) -> {output_dir}")

    mod = types.ModuleType("antenv.axon_hooks")
    mod.get_axon_ntff_profile_hook = lambda: _hook
    mod.set_axon_ntff_profile_hook = lambda h: None
    import antenv

    sys.modules["antenv.axon_hooks"] = mod
    antenv.axon_hooks = mod


def run_traced(sentence, size, W):
    """Like kernel(), but also returns (exec_time_ns, profile_json path)."""
    _install_ntff_hook()
    nc = _get_nc()
    in_maps = _host_inputs(sentence, size, W)
    res = run_bass_kernel_spmd(
        nc, in_maps, core_ids=list(range(NCORES)), trace=True, trace_cores=[0]
    )
    out = np.concatenate([res.results[c]["o"] for c in range(NCORES)], axis=1)
    return out.astype(np.float32), res.exec_time_ns, res.profile_json


if __name__ == "__main__":
    rng = np.random.default_rng(0)
    s = rng.standard_normal((L, B, D)).astype(np.float32)
    sz = rng.integers(0, L, size=(B,)).astype(np.int32)
    W = (rng.standard_normal((D, D)) / np.sqrt(D)).astype(np.float32)
    out = kernel(s, sz, W)
    print("out", out.shape, out.dtype, np.abs(out).max())


# revision 15
# speedup vs baseline: 1.4468x; 1.0688x over previous
"""Trainium2 Bass kernel for nn_BilinearSentenceEncoder (v3, attention-style).

For sentence [L=128, B=4096, D=300], size [B], W [D, D]:
out[l] = w1*s[l] + w0*s[l-1] + w2*s[l+1] with (w0,w1,w2) a masked 3-way
softmax of bilinear scores s_i^T Wsym s_j / D.

v3 structure (per 16-column chunk, per core; B data-parallel over 8 cores):
  s_t  [128l, 16, 384] bf16   <- SWDGE cast load (HBM f32)
  st   [128d, 16, 3, 128l]    <- XBAR transpose of s_t
  per column j:
    v    = st^T @ (Wsym/D)    3 bf16 matmuls -> PSUM [l, 304]
    v_sb = bf16 copy (Act/DVE alternating)
  vt   [128d, 16, 3, 128l]    <- XBAR transpose of v_sb
  per column j:
    A    = st^T @ vt          3 bf16 matmuls -> PSUM [l', l]  (=scores/D)
    E    = exp(A)             Act -> SBUF bf16 [l', l]
    E'   = E zeroed off-band/oversize (copy_predicated, host u8 mask)
    den  = E'^T @ ones        N=1 matmul -> PSUM [l, 1]; r = 1/den (DVE)
    oraw = E'^T @ s           1 matmul N=300 -> PSUM [l, d]
    o    = oraw * r[l]        Act/DVE alternating -> SBUF f32
  store o chunk (DMA).

Column l of E holds exactly the three channel numerators for output l
(E[l-1,l]=e^{w0 logit}, E[l,l]=e^{w1}, E[l+1,l]=e^{w2}); masking zeroes
invalid channels so the normalized column IS the softmax, and padded
positions degenerate to w1=1 (diag never masked).
"""

import sys

sys.path.insert(0, "/opt/trn_rl_repo")

import numpy as np
import ml_dtypes

import concourse.bacc as bacc
import concourse.mybir as mybir
from concourse import tile
from concourse.bass_utils import run_bass_kernel_spmd

dt = mybir.dt
AF = mybir.ActivationFunctionType
ALU = mybir.AluOpType

L, B, D = 128, 4096, 300
NCORES = 8
BC = B // NCORES          # 512 batch columns per core
CHUNK = 16
NCHUNK = BC // CHUNK      # 32
DP = 384                  # d padded to %128 for the XBAR transpose
DCH = [(0, 128), (128, 128), (256, 44)]


def _build_nc():
    nc = bacc.Bacc()
    f32, bf16, u8 = dt.float32, dt.bfloat16, dt.uint8

    s_in = nc.declare_dram_parameter("s", [L, BC, D], f32, isOutput=False)
    wb_in = nc.declare_dram_parameter("wb", [L, 3, 304], bf16, isOutput=False)
    km_in = nc.declare_dram_parameter("km", [128, BC, 128], u8, isOutput=False)
    o_out = nc.declare_dram_parameter("o", [L, BC, D], f32, isOutput=True)

    with tile.TileContext(nc) as tc:
        with (
            tc.tile_pool(name="const", bufs=1) as cpool,
            tc.tile_pool(name="s", bufs=3) as s_pool,
            tc.tile_pool(name="st", bufs=3) as st_pool,
            tc.tile_pool(name="vt", bufs=4) as vt_pool,
            tc.tile_pool(name="km", bufs=2) as km_pool,
            tc.tile_pool(name="e", bufs=2) as e_pool,
            tc.tile_pool(name="r", bufs=2) as r_pool,
            tc.tile_pool(name="o", bufs=2) as o_pool,
            tc.tile_pool(name="vp", bufs=2, space="PSUM") as v_pool,
            tc.tile_pool(name="ap", bufs=2, space="PSUM") as a_pool,
            tc.tile_pool(name="op", bufs=2, space="PSUM") as ops_pool,
        ):
            wb_t = cpool.tile([L, 3, 304], bf16)
            ones_t = cpool.tile([128, 1], bf16)
            zero_t = cpool.tile([128, 1], bf16)
            nc.sync.dma_start(out=wb_t[:, :, :], in_=wb_in[:, :, :])
            nc.vector.memset(ones_t[:, :], 1.0)
            nc.vector.memset(zero_t[:, :], 0.0)

            MCH = [(0, 128), (128, 128), (256, 44)]   # dout chunks
            NQ = NCHUNK * 8                           # batches (2 cols each)
            ctx = {}   # per-chunk tiles, keyed by chunk index

            def emit_loads(c):
                b0 = c * CHUNK
                s_t = s_pool.tile([L, CHUNK, DP], bf16)
                nc.gpsimd.dma_start(out=s_t[:, :, 0:D], in_=s_in[:, b0 : b0 + CHUNK, :])
                # ones column at d=300 feeds the fused den (col 300 of oraw)
                nc.vector.memset(s_t[:, :, 300:301], 1.0)
                km_t = km_pool.tile([128, CHUNK, 128], u8)
                nc.gpsimd.dma_start(out=km_t[:, :, :], in_=km_in[:, b0 : b0 + CHUNK, :])
                # XBAR: st[p, (j,cc), l] = s_t[l, j, 128*cc + p]
                st = st_pool.tile([128, CHUNK, 3, 128], bf16)
                nc.sync.dma_start_transpose(st[:, :, :, :], s_t[:, :, :])
                e_ch = e_pool.tile([128, CHUNK, 128], bf16)
                o_t = o_pool.tile([L, CHUNK, D], f32)
                r_t = r_pool.tile([128, CHUNK], f32)
                ctx[c] = dict(b0=b0, s_t=s_t, km_t=km_t, st=st, e_ch=e_ch,
                              o_t=o_t, r_t=r_t, vt={})

            def emit_vt(qi):
                c, q = qi // 8, qi % 8
                x = ctx[c]
                j0 = 2 * q
                # vt[dout, j, l] = sum_din (Wsym/D)[din, dout] s[l, j, din]
                # moving batched over 2 columns (PSUM-bank limits)
                vt_sb = vt_pool.tile([128, 2, 3, 128], bf16)
                vt_ps = v_pool.tile([128, 3, 2, 128], f32)
                for mi, (m0, mn) in enumerate(MCH):
                    for k, (d0, dn) in enumerate(DCH):
                        nc.tensor.matmul(
                            vt_ps[0:mn, mi, :, :],
                            wb_t[0:dn, k, m0 : m0 + mn],
                            x["st"][0:dn, j0 : j0 + 2, k, :],
                            start=(k == 0),
                            stop=(k == 2),
                        )
                if q % 2 == 0:
                    nc.scalar.activation(
                        vt_sb[:, :, :, :],
                        vt_ps[:, :, :, :].rearrange("p m j l -> p j m l"),
                        AF.Copy,
                    )
                else:
                    nc.vector.tensor_copy(
                        out=vt_sb[:, :, :, :],
                        in_=vt_ps[:, :, :, :].rearrange("p m j l -> p j m l"),
                    )
                x["vt"][q] = vt_sb

            def emit_a(qi):
                c, q = qi // 8, qi % 8
                x = ctx[c]
                j0 = 2 * q
                vt_sb = x["vt"][q]
                for jh in range(2):
                    j = j0 + jh
                    # A[l', l] = sum_dout s[l', dout] vt[dout, l] (scores/D)
                    a_ps = a_pool.tile([128, 128], f32)
                    for k2, (d0, dn) in enumerate(MCH):
                        nc.tensor.matmul(
                            a_ps[:, :],
                            x["st"][0:dn, j, k2, :],
                            vt_sb[0:dn, jh, k2, :],
                            start=(k2 == 0),
                            stop=(k2 == 2),
                        )
                    nc.scalar.activation(x["e_ch"][:, j, :], a_ps[:, :], AF.Exp)
                if q % 4 == 3:
                    # zero non-band / oversize cells for the finished half
                    h0 = 8 * (q // 4)
                    zb = zero_t[:, :].broadcast_to([128, 8, 128])
                    nc.vector.copy_predicated(
                        out=x["e_ch"][:, h0 : h0 + 8, :],
                        mask=x["km_t"][:, h0 : h0 + 8, :],
                        data=zb,
                    )

            def emit_out(qi):
                c, q = qi // 8, qi % 8
                x = ctx[c]
                j0 = 2 * q
                for jh in range(2):
                    j = j0 + jh
                    # oraw[l, 0:300] = sum_l' E'[l', l] s[l', d];  col 300 = den
                    ops = ops_pool.tile([128, 301], f32)
                    nc.tensor.matmul(
                        ops[:, :], x["e_ch"][:, j, :], x["s_t"][:, j, 0:301],
                        start=True, stop=True,
                    )
                    nc.vector.reciprocal(x["r_t"][:, j : j + 1], ops[:, 300:301])
                    if j % 2 == 0:
                        nc.scalar.activation(
                            x["o_t"][:, j, :], ops[:, 0:300], AF.Copy,
                            scale=x["r_t"][:, j : j + 1],
                        )
                    else:
                        nc.vector.tensor_scalar(
                            out=x["o_t"][:, j, :], in0=ops[:, 0:300],
                            scalar1=x["r_t"][:, j : j + 1], scalar2=None,
                            op0=ALU.mult,
                        )
                if q == 7:
                    nc.sync.dma_start(
                        out=o_out[:, x["b0"] : x["b0"] + CHUNK, :],
                        in_=x["o_t"][:, :, :],
                    )
                    del ctx[c]

            # flat software pipeline over quarters: vt(qi) | A+exp(qi-1) |
            # out(qi-3); A(q) streams on the PE while Act copies vt_sb(q+1),
            # out lags far enough for exp+mask of its half to finish.
            for qi in range(NQ + 4):
                if qi < NQ:
                    if qi % 8 == 0:
                        emit_loads(qi // 8)
                    emit_vt(qi)
                if 1 <= qi < NQ + 1:
                    emit_a(qi - 1)
                if qi >= 4:
                    emit_out(qi - 4)

    nc.compile()
    return nc


_NC_CACHE = {}


def _get_nc():
    if "nc" not in _NC_CACHE:
        _NC_CACHE["nc"] = _build_nc()
    return _NC_CACHE["nc"]


def _host_inputs(sentence, size, W):
    sentence = np.ascontiguousarray(np.asarray(sentence, dtype=np.float32))
    size = np.asarray(size).astype(np.int64)
    W = np.asarray(W, dtype=np.float32)

    wsym = 0.5 * (W + W.T) / np.float32(D)
    wb = np.zeros((128, 3, 304), dtype=ml_dtypes.bfloat16)
    for i, (d0, dn) in enumerate(DCH):
        wb[0:dn, i, 0:D] = wsym[d0 : d0 + dn, :].astype(ml_dtypes.bfloat16)

    # kill-mask km[l', b, l] = 1 where E must be zeroed.
    # allowed cells per output column l: (l, l) always;
    # (l-1, l) iff l >= 1 and l < size_b; (l+1, l) iff l <= 126 and l < size_b - 1.
    lp = np.arange(128)[:, None, None]          # l'
    lc = np.arange(128)[None, None, :]          # l
    sz = size[None, :, None].astype(np.int64)   # b
    allow = (lp == lc)
    allow = allow | ((lp == lc - 1) & (lc < sz))
    allow = allow | ((lp == lc + 1) & (lc < sz - 1))
    km_full = (~allow).astype(np.uint8)         # [128, B, 128]

    in_maps = []
    for c in range(NCORES):
        bsl = slice(c * BC, (c + 1) * BC)
        in_maps.append(
            {
                "s": np.ascontiguousarray(sentence[:, bsl, :]),
                "wb": wb,
                "km": np.ascontiguousarray(km_full[:, bsl, :]),
            }
        )
    return in_maps


def kernel(sentence, size, W):
    nc = _get_nc()
    in_maps = _host_inputs(sentence, size, W)
    res = run_bass_kernel_spmd(nc, in_maps, core_ids=list(range(NCORES)))
    out = np.concatenate([res.results[c]["o"] for c in range(NCORES)], axis=1)
    return out.astype(np.float32)


def _install_ntff_hook():
    """Register the axon NTFF profiling hook that this container's boot
    skipped (antenv.axon_hooks module absent)."""
    try:
        from antenv.axon_hooks import get_axon_ntff_profile_hook  # noqa: F401

        return
    except ImportError:
        pass
    import contextlib
    import ctypes
    import types

    so_path = "/opt/axon/libaxon_pjrt.so"
    lib = ctypes.CDLL(so_path)
    if not hasattr(lib, "axon_start_nrt_profile"):
        return
    lib.axon_start_nrt_profile.argtypes = [
        ctypes.POINTER(ctypes.c_int64),
        ctypes.c_size_t,
    ]
    lib.axon_start_nrt_profile.restype = ctypes.c_int64
    lib.axon_stop_nrt_profile.argtypes = [ctypes.c_char_p]
    lib.axon_stop_nrt_profile.restype = ctypes.c_int64

    @contextlib.contextmanager
    def _hook(output_dir, device_ids):
        import jax

        jax.devices()
        if device_ids:
            ids = (ctypes.c_int64 * len(device_ids))(*device_ids)
            rc = lib.axon_start_nrt_profile(ids, len(device_ids))
        else:
            rc = lib.axon_start_nrt_profile(None, 0)
        if rc != 0:
            raise RuntimeError(f"axon_start_nrt_profile rc={rc}")
        try:
            yield
        finally:
            n = lib.axon_stop_nrt_profile(str(output_dir).encode())
            print(f"ntff capture: {n} file(s) -> {output_dir}")

    mod = types.ModuleType("antenv.axon_hooks")
    mod.get_axon_ntff_profile_hook = lambda: _hook
    mod.set_axon_ntff_profile_hook = lambda h: None
    import antenv

    sys.modules["antenv.axon_hooks"] = mod
    antenv.axon_hooks = mod


def run_traced(sentence, size, W):
    """Like kernel(), but also returns (exec_time_ns, profile_json path)."""
    _install_ntff_hook()
    nc = _get_nc()
    in_maps = _host_inputs(sentence, size, W)
    res = run_bass_kernel_spmd(
        nc, in_maps, core_ids=list(range(NCORES)), trace=True, trace_cores=[0]
    )
    out = np.concatenate([res.results[c]["o"] for c in range(NCORES)], axis=1)
    return out.astype(np.float32), res.exec_time_ns, res.profile_json


if __name__ == "__main__":
    rng = np.random.default_rng(0)
    s = rng.standard_normal((L, B, D)).astype(np.float32)
    sz = rng.integers(0, L, size=(B,)).astype(np.int32)
    W = (rng.standard_normal((D, D)) / np.sqrt(D)).astype(np.float32)
    out = kernel(s, sz, W)
    print("out", out.shape, out.dtype, np.abs(out).max())
